# revision 5
# baseline (speedup 1.0000x reference)
"""Trainium2 Bass kernel for windowed (block-local) multi-head attention:
  y = LayerNorm(x + Proj(Attn(QKV(x))))  with non-overlapping windows of 64.

Sharding: B*L = 16384 tokens are split into 8 contiguous shards of 2048
tokens (window- and batch-aligned), one per NeuronCore; weights replicated.
Each core runs the full fused pipeline on its shard:

  phase 1: qT/kT (feature-major, via host-transposed x), V (token-major,
           with a fused ones-column per head), per-window softmax
           (scoresT layout; expT serves directly as the AV matmul's lhsT so
           no per-window transposes are needed; row-sums come from the ones
           column), block-transpose of the attention output to feature-major,
           bounced to an HBM scratch buffer.
  phase 2: output projection (fp32r), residual add, LayerNorm, store.

Projection matmuls run in fp32r (TF32-like, full PE speed at N>=256);
attention matmuls run in plain fp32 (same cost at N<=65, exact).
"""

import os
import sys
import types

for _p in ("/opt/trn_rl_repo", "/root/.axon_site/_ro/trn_rl_repo"):
    if os.path.isdir(_p) and _p not in sys.path:
        sys.path.append(_p)

import numpy as np

import concourse.bass as bass
import concourse.bacc as bacc
import concourse.mybir as mybir
import concourse.tile as tile
from concourse.bass_utils import run_bass_kernel_spmd
from concourse.masks import make_identity

# Problem shape (hardcoded per contest contract).
B, L, D = 4, 4096, 1024
H, W, HD = 16, 64, 64
EPS = 1e-5
NCORES = 8
TOK = (B * L) // NCORES        # tokens per core = 2048
T = 256                        # token block per pipeline iteration
NBLK = TOK // T                # 8
KT = D // 128                  # 8 contraction tiles
NWIN = T // W                  # 4 windows per block
VC = H * (HD + 1)              # V store width with ones columns = 1040
F32, F32R = mybir.dt.float32, mybir.dt.float32r
AF = mybir.ActivationFunctionType


def _maybe_install_ntff_hook():
    """The image's antenv lacks axon_hooks; synthesize it so trace=True works."""
    if "antenv.axon_hooks" in sys.modules:
        return
    try:
        import ctypes
        from trn_agent_boot.trn_boot import _ntff_profile_via_ctypes
        so = "/opt/axon/libaxon_pjrt.so"
        if hasattr(ctypes.CDLL(so), "axon_start_nrt_profile"):
            m = types.ModuleType("antenv.axon_hooks")
            hook = _ntff_profile_via_ctypes(so)
            m.get_axon_ntff_profile_hook = lambda: hook
            m.set_axon_ntff_profile_hook = lambda h: None
            sys.modules["antenv.axon_hooks"] = m
    except Exception:
        pass


def _build(has_bqk, has_bv, has_bp, has_gamma, has_beta):
    nc = bacc.Bacc("TRN2", target_bir_lowering=False, debug=False)

    xT = nc.dram_tensor("xT", [D, TOK], F32R, kind="ExternalInput")
    xr = nc.dram_tensor("xr", [TOK, D], F32, kind="ExternalInput")
    wqk = nc.dram_tensor("wqk", [D, 2 * D], F32R, kind="ExternalInput")
    wv = nc.dram_tensor("wv", [D, VC], F32R, kind="ExternalInput")
    wp = nc.dram_tensor("wp", [D, D], F32R, kind="ExternalInput")
    bqk = nc.dram_tensor("bqk", [128, 2 * KT], F32, kind="ExternalInput") if has_bqk else None
    bv = nc.dram_tensor("bv", [1, VC], F32, kind="ExternalInput") if has_bv else None
    bp = nc.dram_tensor("bp", [1, D], F32, kind="ExternalInput") if has_bp else None
    gamma = nc.dram_tensor("gamma", [1, D], F32, kind="ExternalInput") if has_gamma else None
    beta = nc.dram_tensor("beta", [1, D], F32, kind="ExternalInput") if has_beta else None
    aT = nc.dram_tensor("aT", [D, TOK], F32)  # internal scratch (feature-major attn out)
    out = nc.dram_tensor("out", [TOK, D], F32, kind="ExternalOutput")

    xT_v = xT.ap().rearrange("(kt p) t -> p kt t", p=128)
    wqk_v = wqk.ap().rearrange("(kt p) m -> p kt m", p=128)
    wv_v = wv.ap().rearrange("(kt p) m -> p kt m", p=128)
    wp_v = wp.ap().rearrange("(kt p) m -> p kt m", p=128)
    aTw_v = aT.ap()                                       # [D, TOK] f32 for writes
    aTr_v = aT.ap().bitcast(F32R).rearrange("(kt p) t -> p kt t", p=128)
    xr_v = xr.ap().rearrange("(b s p) d -> b p s d", p=128, s=2)  # [NBLK,128,2,D]

    # ---------------- phase 1: QKV + attention ----------------
    with tile.TileContext(nc) as tc:
        with (
            tc.tile_pool(name="consts", bufs=1) as consts,
            tc.tile_pool(name="xt", bufs=2) as xt_pool,
            tc.tile_pool(name="qk", bufs=1) as qk_pool,
            tc.tile_pool(name="vst", bufs=2) as v_pool,
            tc.tile_pool(name="exp", bufs=4) as exp_pool,
            tc.tile_pool(name="rs", bufs=4) as rs_pool,
            tc.tile_pool(name="ao", bufs=3) as ao_pool,
            tc.tile_pool(name="aot", bufs=3) as aot_pool,
            tc.tile_pool(name="mmps", bufs=2, space="PSUM") as mm_ps,
            tc.tile_pool(name="scps", bufs=3, space="PSUM") as sc_ps,
            tc.tile_pool(name="avps", bufs=2, space="PSUM") as av_ps,
            tc.tile_pool(name="tpps", bufs=1, space="PSUM") as tp_ps,
        ):
            wqk_sb = consts.tile([128, KT, 2 * D], F32R)
            nc.sync.dma_start(out=wqk_sb, in_=wqk_v)
            wv_sb = consts.tile([128, KT, VC], F32R)
            nc.sync.dma_start(out=wv_sb, in_=wv_v)
            ident = consts.tile([128, 128], F32)
            make_identity(nc, ident)
            if has_bqk:
                bqk_sb = consts.tile([128, 2 * KT], F32)
                nc.sync.dma_start(out=bqk_sb, in_=bqk.ap())
            if has_bv:
                bv_sb = consts.tile([128, VC], F32)
                nc.sync.dma_start(out=bv_sb, in_=bv.ap().to_broadcast((128, VC)))

            for blk in range(NBLK):
                ts0 = blk * T
                xt_t = xt_pool.tile([128, KT, T], F32R)
                nc.sync.dma_start(out=xt_t, in_=xT_v[:, :, ts0:ts0 + T])

                # qT/kT feature-major: 16 m-blocks of 128 features
                qk_t = []
                for mb in range(2 * KT):
                    ps = mm_ps.tile([128, T], F32, name="mmq", tag="vps")
                    for kt in range(KT):
                        nc.tensor.matmul(
                            ps,
                            wqk_sb[:, kt, mb * 128:(mb + 1) * 128],
                            xt_t[:, kt, :],
                            start=(kt == 0), stop=(kt == KT - 1),
                        )
                    qkt = qk_pool.tile([128, T], F32, name=f"qk{mb}", tag=f"qk{mb}")
                    if has_bqk:
                        nc.scalar.activation(qkt, ps, AF.Identity,
                                             bias=bqk_sb[:, mb:mb + 1])
                    else:
                        nc.scalar.copy(qkt, ps)
                    qk_t.append(qkt)

                # V token-major with ones columns: [128, 2, 1040]
                v_t = v_pool.tile([128, 2, VC], F32)
                NVB, VB = 4, VC // 4                      # 4 x 260
                for tsub in range(2):
                    for nb in range(NVB):
                        ps = mm_ps.tile([128, VB], F32, name="mmv", tag="vps")
                        for kt in range(KT):
                            nc.tensor.matmul(
                                ps,
                                xt_t[:, kt, tsub * 128:(tsub + 1) * 128],
                                wv_sb[:, kt, nb * VB:(nb + 1) * VB],
                                start=(kt == 0), stop=(kt == KT - 1),
                            )
                        dst = v_t[:, tsub, nb * VB:(nb + 1) * VB]
                        if has_bv:
                            nc.vector.tensor_add(dst, ps, bv_sb[:, nb * VB:(nb + 1) * VB])
                        else:
                            nc.vector.tensor_copy(dst, ps)
                    ones_cols = v_t[:, tsub, :].rearrange(
                        "p (h c) -> p h c", c=HD + 1)[:, :, HD:HD + 1]
                    nc.vector.memset(ones_cols, 1.0)

                # windowed attention; everything for window w lives at
                # partition offset o = (w % 2) * 64
                ao_t = [None, None]
                for w in range(NWIN):
                    o = (w % 2) * 64
                    tsub = w // 2
                    if w % 2 == 0:
                        ao_t[tsub] = ao_pool.tile([128, D], F32, name="ao", tag="ao")
                    wc = slice(w * W, (w + 1) * W)
                    for h in range(H):
                        qr = (h % 2) * 64
                        kt_t = qk_t[KT + h // 2]          # kT features
                        qt_t = qk_t[h // 2]               # qT features
                        sps = sc_ps.tile([128, W], F32, name="sps", tag="sps")
                        nc.tensor.matmul(
                            sps[o:o + W, :],
                            kt_t[qr:qr + HD, wc],
                            qt_t[qr:qr + HD, wc],
                            start=True, stop=True,
                        )
                        ex = exp_pool.tile([128, W], F32, name="ex", tag="ex")
                        nc.scalar.activation(ex[o:o + W, :], sps[o:o + W, :], AF.Exp)
                        aps = av_ps.tile([128, HD + 1], F32, name="aps", tag="aps")
                        nc.tensor.matmul(
                            aps[o:o + W, :],
                            ex[o:o + W, :],
                            v_t[o:o + W, tsub, h * (HD + 1):(h + 1) * (HD + 1)],
                            start=True, stop=True,
                        )
                        rs = rs_pool.tile([128, 1], F32, name="rsum", tag="rsum")
                        nc.vector.reciprocal(rs[o:o + W, :], aps[o:o + W, HD:HD + 1])
                        nc.vector.tensor_scalar_mul(
                            ao_t[tsub][o:o + W, h * HD:(h + 1) * HD],
                            in0=aps[o:o + W, :HD],
                            scalar1=rs[o:o + W, :],
                        )

                # transpose attention output to feature-major, bounce to HBM
                for cb in range(KT):
                    aot = aot_pool.tile([128, T], F32, name="aot", tag="aot")
                    for tsub in range(2):
                        tps = tp_ps.tile([128, 128], F32, name="tps", tag="tps")
                        nc.tensor.transpose(
                            tps, ao_t[tsub][:, cb * 128:(cb + 1) * 128], ident)
                        nc.vector.tensor_copy(
                            aot[:, tsub * 128:(tsub + 1) * 128], tps)
                    nc.sync.dma_start(
                        out=aTw_v[cb * 128:(cb + 1) * 128, ts0:ts0 + T], in_=aot)

    # ---------------- phase 2: proj + residual + LayerNorm ----------------
    with tile.TileContext(nc) as tc:
        with (
            tc.tile_pool(name="consts2", bufs=1) as consts2,
            tc.tile_pool(name="at", bufs=2) as at_pool,
            tc.tile_pool(name="xres", bufs=2) as xr_pool,
            tc.tile_pool(name="yy", bufs=3) as y_pool,
            tc.tile_pool(name="st", bufs=4) as st_pool,
            tc.tile_pool(name="ps2", bufs=4, space="PSUM") as ps2,
        ):
            wp_sb = consts2.tile([128, KT, D], F32R)
            nc.sync.dma_start(out=wp_sb, in_=wp_v)
            eps_t = consts2.tile([128, 1], F32)
            nc.vector.memset(eps_t, EPS)
            if has_bp:
                bp_sb = consts2.tile([128, D], F32)
                nc.sync.dma_start(out=bp_sb, in_=bp.ap().to_broadcast((128, D)))
            if has_gamma:
                g_sb = consts2.tile([128, D], F32)
                nc.sync.dma_start(out=g_sb, in_=gamma.ap().to_broadcast((128, D)))
            if has_beta:
                be_sb = consts2.tile([128, D], F32)
                nc.sync.dma_start(out=be_sb, in_=beta.ap().to_broadcast((128, D)))

            SG = 2                                        # bn_stats subgroups (512 each)
            for blk in range(NBLK):
                ts0 = blk * T
                at_t = at_pool.tile([128, KT, T], F32R)
                nc.sync.dma_start(out=at_t, in_=aTr_v[:, :, ts0:ts0 + T])
                xr_t = xr_pool.tile([128, 2, D], F32)
                nc.sync.dma_start(out=xr_t, in_=xr_v[blk])

                for tsub in range(2):
                    y_t = y_pool.tile([128, D], F32, name="yt", tag="yt")
                    for nb in range(2):
                        ps = ps2.tile([128, 512], F32, name="pps", tag="pps")
                        for kt in range(KT):
                            nc.tensor.matmul(
                                ps,
                                at_t[:, kt, tsub * 128:(tsub + 1) * 128],
                                wp_sb[:, kt, nb * 512:(nb + 1) * 512],
                                start=(kt == 0), stop=(kt == KT - 1),
                            )
                        dst = y_t[:, nb * 512:(nb + 1) * 512]
                        nc.vector.tensor_add(dst, ps, xr_t[:, tsub, nb * 512:(nb + 1) * 512])
                    if has_bp:
                        nc.vector.tensor_add(y_t, y_t, bp_sb)

                    stats = st_pool.tile([128, SG, 6], F32, name="stats", tag="stats")
                    for sg in range(SG):
                        nc.vector.bn_stats(out=stats[:, sg, :],
                                           in_=y_t[:, sg * 512:(sg + 1) * 512])
                    mv = st_pool.tile([128, 2], F32, name="mv", tag="mv")
                    nc.vector.bn_aggr(out=mv, in_=stats)
                    nc.scalar.activation(out=mv[:, 1:2], in_=mv[:, 1:2],
                                         func=AF.Sqrt, bias=eps_t)
                    nc.vector.reciprocal(mv[:, 1:2], mv[:, 1:2])
                    nc.vector.tensor_scalar(
                        out=y_t, in0=y_t,
                        scalar1=mv[:, 0:1], scalar2=mv[:, 1:2],
                        op0=mybir.AluOpType.subtract, op1=mybir.AluOpType.mult,
                    )
                    if has_gamma:
                        nc.vector.tensor_mul(y_t, y_t, g_sb)
                    if has_beta:
                        nc.vector.tensor_add(y_t, y_t, be_sb)
                    nc.sync.dma_start(
                        out=out.ap()[ts0 + tsub * 128:ts0 + (tsub + 1) * 128, :],
                        in_=y_t)

    nc.compile()
    return nc


_CACHE = {}


def _get_nc(flags):
    if flags not in _CACHE:
        _CACHE[flags] = _build(*flags)
    return _CACHE[flags]


def kernel(x, Wqkv, bqkv, Wproj, bproj, gamma, beta):
    x = np.ascontiguousarray(np.asarray(x, dtype=np.float32))
    Wqkv = np.asarray(Wqkv, dtype=np.float32)
    bqkv = np.asarray(bqkv, dtype=np.float32)
    Wproj = np.asarray(Wproj, dtype=np.float32)
    bproj = np.asarray(bproj, dtype=np.float32)
    gamma = np.asarray(gamma, dtype=np.float32)
    beta = np.asarray(beta, dtype=np.float32)

    scale = np.float32(HD ** -0.5)
    # fold the attention scale into Wq / bq
    svec = np.ones((2 * D,), np.float32)
    svec[:D] = scale
    wqk_h = np.ascontiguousarray((Wqkv[:2 * D] * svec[:, None]).T)   # [D, 2D]
    bqk_full = bqkv[:2 * D] * svec
    # V weights with a zero column per head (ones come from an on-chip memset)
    wv_h = np.zeros((D, VC), np.float32)
    bv_full = np.zeros((VC,), np.float32)
    wvT = Wqkv[2 * D:].T                                             # [D, D]
    for h in range(H):
        wv_h[:, h * (HD + 1):h * (HD + 1) + HD] = wvT[:, h * HD:(h + 1) * HD]
        bv_full[h * (HD + 1):h * (HD + 1) + HD] = bqkv[2 * D + h * HD:2 * D + (h + 1) * HD]
    wp_h = np.ascontiguousarray(Wproj.T)                             # [D, D]

    has_bqk = bool(np.any(bqk_full))
    has_bv = bool(np.any(bv_full))
    has_bp = bool(np.any(bproj))
    has_gamma = not bool(np.all(gamma == 1.0))
    has_beta = bool(np.any(beta))
    nc = _get_nc((has_bqk, has_bv, has_bp, has_gamma, has_beta))

    xf = x.reshape(B * L, D)
    in_maps = []
    for c in range(NCORES):
        sl = slice(c * TOK, (c + 1) * TOK)
        m = {
            "xT": np.ascontiguousarray(xf[sl].T),
            "xr": np.ascontiguousarray(xf[sl]),
            "wqk": wqk_h,
            "wv": wv_h,
            "wp": wp_h,
        }
        if has_bqk:
            m["bqk"] = np.ascontiguousarray(bqk_full.reshape(2 * KT, 128).T)
        if has_bv:
            m["bv"] = bv_full.reshape(1, VC)
        if has_bp:
            m["bp"] = bproj.reshape(1, D)
        if has_gamma:
            m["gamma"] = gamma.reshape(1, D)
        if has_beta:
            m["beta"] = beta.reshape(1, D)
        in_maps.append(m)

    trace = bool(os.environ.get("KERNEL_TRACE"))
    if trace:
        _maybe_install_ntff_hook()
    res = run_bass_kernel_spmd(nc, in_maps, core_ids=list(range(NCORES)),
                               trace=trace)
    if trace:
        kernel.last_exec_time_ns = res.exec_time_ns
        kernel.last_mean_exec_time_ns = res.mean_exec_time_ns
        kernel.last_results = res

    y = np.concatenate([res.results[c]["out"] for c in range(NCORES)], axis=0)
    return y.reshape(B, L, D)


# revision 7
# speedup vs baseline: 1.1162x; 1.1162x over previous
"""Trainium2 Bass kernel for windowed (block-local) multi-head attention:
  y = LayerNorm(x + Proj(Attn(QKV(x))))  with non-overlapping windows of 64.

Sharding: B*L = 16384 tokens are split into 8 contiguous shards of 2048
tokens (window- and batch-aligned), one per NeuronCore; weights replicated.
Each core runs the full fused pipeline on its shard:

  phase 1: qT/kT (feature-major, via host-transposed x), V (token-major,
           with a fused ones-column per head), per-window softmax
           (scoresT layout; expT serves directly as the AV matmul's lhsT so
           no per-window transposes are needed; row-sums come from the ones
           column), block-transpose of the attention output to feature-major,
           bounced to an HBM scratch buffer.
  phase 2: output projection (fp32r), residual add, LayerNorm, store.

Projection matmuls run in fp32r (TF32-like, full PE speed at N>=256);
attention matmuls run in plain fp32 (same cost at N<=65, exact).
"""

import os
import sys
import types

for _p in ("/opt/trn_rl_repo", "/root/.axon_site/_ro/trn_rl_repo"):
    if os.path.isdir(_p) and _p not in sys.path:
        sys.path.append(_p)

import numpy as np

import concourse.bass as bass
import concourse.bacc as bacc
import concourse.mybir as mybir
import concourse.tile as tile
from concourse.bass_utils import run_bass_kernel_spmd
from concourse.masks import make_identity

# Problem shape (hardcoded per contest contract).
B, L, D = 4, 4096, 1024
H, W, HD = 16, 64, 64
EPS = 1e-5
NCORES = 8
TOK = (B * L) // NCORES        # tokens per core = 2048
T = 256                        # token block per pipeline iteration
NBLK = TOK // T                # 8
KT = D // 128                  # 8 contraction tiles
NWIN = T // W                  # 4 windows per block
VC = H * (HD + 1)              # V store width with ones columns = 1040
F32, F32R, BF16 = mybir.dt.float32, mybir.dt.float32r, mybir.dt.bfloat16
ATT_DT = BF16
AF = mybir.ActivationFunctionType


def _maybe_install_ntff_hook():
    """The image's antenv lacks axon_hooks; synthesize it so trace=True works."""
    if "antenv.axon_hooks" in sys.modules:
        return
    try:
        import ctypes
        from trn_agent_boot.trn_boot import _ntff_profile_via_ctypes
        so = "/opt/axon/libaxon_pjrt.so"
        if hasattr(ctypes.CDLL(so), "axon_start_nrt_profile"):
            m = types.ModuleType("antenv.axon_hooks")
            hook = _ntff_profile_via_ctypes(so)
            m.get_axon_ntff_profile_hook = lambda: hook
            m.set_axon_ntff_profile_hook = lambda h: None
            sys.modules["antenv.axon_hooks"] = m
    except Exception:
        pass


def _build(has_bqk, has_bv, has_bp, has_gamma, has_beta):
    nc = bacc.Bacc("TRN2", target_bir_lowering=False, debug=False)

    xT = nc.dram_tensor("xT", [D, TOK], F32R, kind="ExternalInput")
    xr = nc.dram_tensor("xr", [TOK, D], F32, kind="ExternalInput")
    wqk = nc.dram_tensor("wqk", [D, 2 * D], F32R, kind="ExternalInput")
    wv = nc.dram_tensor("wv", [D, VC], F32R, kind="ExternalInput")
    wp = nc.dram_tensor("wp", [D, D], F32R, kind="ExternalInput")
    bqk = nc.dram_tensor("bqk", [128, 2 * KT], F32, kind="ExternalInput") if has_bqk else None
    bv = nc.dram_tensor("bv", [1, VC], F32, kind="ExternalInput") if has_bv else None
    bp = nc.dram_tensor("bp", [1, D], F32, kind="ExternalInput") if has_bp else None
    gamma = nc.dram_tensor("gamma", [1, D], F32, kind="ExternalInput") if has_gamma else None
    beta = nc.dram_tensor("beta", [1, D], F32, kind="ExternalInput") if has_beta else None
    aT = nc.dram_tensor("aT", [D, TOK], F32)  # internal scratch (feature-major attn out)
    out = nc.dram_tensor("out", [TOK, D], F32, kind="ExternalOutput")

    xT_v = xT.ap().rearrange("(kt p) t -> p kt t", p=128)
    wqk_v = wqk.ap().rearrange("(kt p) m -> p kt m", p=128)
    wv_v = wv.ap().rearrange("(kt p) m -> p kt m", p=128)
    wp_v = wp.ap().rearrange("(kt p) m -> p kt m", p=128)
    aTw_v = aT.ap()                                       # [D, TOK] f32 for writes
    aTr_v = aT.ap().bitcast(F32R).rearrange("(kt p) t -> p kt t", p=128)
    xr_v = xr.ap().rearrange("(b s p) d -> b p s d", p=128, s=2)  # [NBLK,128,2,D]

    # ---------------- phase 1: QKV + attention ----------------
    with tile.TileContext(nc) as tc:
        with (
            tc.tile_pool(name="consts", bufs=1) as consts,
            tc.tile_pool(name="xt", bufs=2) as xt_pool,
            tc.tile_pool(name="qk", bufs=1) as qk_pool,
            tc.tile_pool(name="vst", bufs=2) as v_pool,
            tc.tile_pool(name="exp", bufs=4) as exp_pool,
            tc.tile_pool(name="rs", bufs=4) as rs_pool,
            tc.tile_pool(name="ao", bufs=3) as ao_pool,
            tc.tile_pool(name="aot", bufs=3) as aot_pool,
            tc.tile_pool(name="mmps", bufs=2, space="PSUM") as mm_ps,
            tc.tile_pool(name="scps", bufs=3, space="PSUM") as sc_ps,
            tc.tile_pool(name="avps", bufs=2, space="PSUM") as av_ps,
            tc.tile_pool(name="tpps", bufs=1, space="PSUM") as tp_ps,
        ):
            wqk_sb = consts.tile([128, KT, 2 * D], F32R)
            nc.sync.dma_start(out=wqk_sb, in_=wqk_v)
            wv_sb = consts.tile([128, KT, VC], F32R)
            nc.sync.dma_start(out=wv_sb, in_=wv_v)
            ident = consts.tile([128, 128], F32)
            make_identity(nc, ident)
            if has_bqk:
                bqk_sb = consts.tile([128, 2 * KT], F32)
                nc.sync.dma_start(out=bqk_sb, in_=bqk.ap())
            if has_bv:
                bv_sb = consts.tile([128, VC], F32)
                nc.sync.dma_start(out=bv_sb, in_=bv.ap().to_broadcast((128, VC)))

            for blk in range(NBLK):
                ts0 = blk * T
                xt_t = xt_pool.tile([128, KT, T], F32R)
                nc.sync.dma_start(out=xt_t, in_=xT_v[:, :, ts0:ts0 + T])

                # qT/kT feature-major: 16 m-blocks of 128 features
                qk_t = []
                for mb in range(2 * KT):
                    ps = mm_ps.tile([128, T], F32, name="mmq", tag="vps")
                    for kt in range(KT):
                        nc.tensor.matmul(
                            ps,
                            wqk_sb[:, kt, mb * 128:(mb + 1) * 128],
                            xt_t[:, kt, :],
                            start=(kt == 0), stop=(kt == KT - 1),
                        )
                    qkt = qk_pool.tile([128, T], ATT_DT, name=f"qk{mb}", tag=f"qk{mb}")
                    if has_bqk:
                        nc.scalar.activation(qkt, ps, AF.Identity,
                                             bias=bqk_sb[:, mb:mb + 1])
                    else:
                        nc.scalar.copy(qkt, ps)
                    qk_t.append(qkt)

                # V token-major with ones columns: [128, 2, 1040]
                v_t = v_pool.tile([128, 2, VC], ATT_DT)
                NVB, VB = 4, VC // 4                      # 4 x 260
                for tsub in range(2):
                    for nb in range(NVB):
                        ps = mm_ps.tile([128, VB], F32, name="mmv", tag="vps")
                        for kt in range(KT):
                            nc.tensor.matmul(
                                ps,
                                xt_t[:, kt, tsub * 128:(tsub + 1) * 128],
                                wv_sb[:, kt, nb * VB:(nb + 1) * VB],
                                start=(kt == 0), stop=(kt == KT - 1),
                            )
                        dst = v_t[:, tsub, nb * VB:(nb + 1) * VB]
                        if has_bv:
                            nc.vector.tensor_add(dst, ps, bv_sb[:, nb * VB:(nb + 1) * VB])
                        else:
                            nc.vector.tensor_copy(dst, ps)
                    ones_cols = v_t[:, tsub, :].rearrange(
                        "p (h c) -> p h c", c=HD + 1)[:, :, HD:HD + 1]
                    nc.vector.memset(ones_cols, 1.0)

                # windowed attention; everything for window w lives at
                # partition offset o = (w % 2) * 64
                ao_t = [None, None]
                for w in range(NWIN):
                    o = (w % 2) * 64
                    tsub = w // 2
                    if w % 2 == 0:
                        ao_t[tsub] = ao_pool.tile([128, D], F32, name="ao", tag="ao")
                    wc = slice(w * W, (w + 1) * W)
                    for h in range(H):
                        qr = (h % 2) * 64
                        kt_t = qk_t[KT + h // 2]          # kT features
                        qt_t = qk_t[h // 2]               # qT features
                        sps = sc_ps.tile([128, W], F32, name="sps", tag="sps")
                        nc.tensor.matmul(
                            sps[o:o + W, :],
                            kt_t[qr:qr + HD, wc],
                            qt_t[qr:qr + HD, wc],
                            start=True, stop=True,
                        )
                        ex = exp_pool.tile([128, W], ATT_DT, name="ex", tag="ex")
                        nc.scalar.activation(ex[o:o + W, :], sps[o:o + W, :], AF.Exp)
                        aps = av_ps.tile([128, HD + 1], F32, name="aps", tag="aps")
                        nc.tensor.matmul(
                            aps[o:o + W, :],
                            ex[o:o + W, :],
                            v_t[o:o + W, tsub, h * (HD + 1):(h + 1) * (HD + 1)],
                            start=True, stop=True,
                        )
                        rs = rs_pool.tile([128, 1], F32, name="rsum", tag="rsum")
                        nc.vector.reciprocal(rs[o:o + W, :], aps[o:o + W, HD:HD + 1])
                        nc.vector.tensor_scalar_mul(
                            ao_t[tsub][o:o + W, h * HD:(h + 1) * HD],
                            in0=aps[o:o + W, :HD],
                            scalar1=rs[o:o + W, :],
                        )

                # transpose attention output to feature-major, bounce to HBM
                for cb in range(KT):
                    aot = aot_pool.tile([128, T], F32, name="aot", tag="aot")
                    for tsub in range(2):
                        tps = tp_ps.tile([128, 128], F32, name="tps", tag="tps")
                        nc.tensor.transpose(
                            tps, ao_t[tsub][:, cb * 128:(cb + 1) * 128], ident)
                        nc.vector.tensor_copy(
                            aot[:, tsub * 128:(tsub + 1) * 128], tps)
                    nc.sync.dma_start(
                        out=aTw_v[cb * 128:(cb + 1) * 128, ts0:ts0 + T], in_=aot)

    # ---------------- phase 2: proj + residual + LayerNorm ----------------
    with tile.TileContext(nc) as tc:
        with (
            tc.tile_pool(name="consts2", bufs=1) as consts2,
            tc.tile_pool(name="at", bufs=2) as at_pool,
            tc.tile_pool(name="xres", bufs=2) as xr_pool,
            tc.tile_pool(name="yy", bufs=3) as y_pool,
            tc.tile_pool(name="st", bufs=4) as st_pool,
            tc.tile_pool(name="ps2", bufs=4, space="PSUM") as ps2,
        ):
            wp_sb = consts2.tile([128, KT, D], F32R)
            nc.sync.dma_start(out=wp_sb, in_=wp_v)
            eps_t = consts2.tile([128, 1], F32)
            nc.vector.memset(eps_t, EPS)
            if has_bp:
                bp_sb = consts2.tile([128, D], F32)
                nc.sync.dma_start(out=bp_sb, in_=bp.ap().to_broadcast((128, D)))
            if has_gamma:
                g_sb = consts2.tile([128, D], F32)
                nc.sync.dma_start(out=g_sb, in_=gamma.ap().to_broadcast((128, D)))
            if has_beta:
                be_sb = consts2.tile([128, D], F32)
                nc.sync.dma_start(out=be_sb, in_=beta.ap().to_broadcast((128, D)))

            SG = 2                                        # bn_stats subgroups (512 each)
            for blk in range(NBLK):
                ts0 = blk * T
                at_t = at_pool.tile([128, KT, T], F32R)
                nc.sync.dma_start(out=at_t, in_=aTr_v[:, :, ts0:ts0 + T])
                xr_t = xr_pool.tile([128, 2, D], F32)
                nc.sync.dma_start(out=xr_t, in_=xr_v[blk])

                for tsub in range(2):
                    y_t = y_pool.tile([128, D], F32, name="yt", tag="yt")
                    for nb in range(2):
                        ps = ps2.tile([128, 512], F32, name="pps", tag="pps")
                        for kt in range(KT):
                            nc.tensor.matmul(
                                ps,
                                at_t[:, kt, tsub * 128:(tsub + 1) * 128],
                                wp_sb[:, kt, nb * 512:(nb + 1) * 512],
                                start=(kt == 0), stop=(kt == KT - 1),
                            )
                        dst = y_t[:, nb * 512:(nb + 1) * 512]
                        nc.vector.tensor_add(dst, ps, xr_t[:, tsub, nb * 512:(nb + 1) * 512])
                    if has_bp:
                        nc.vector.tensor_add(y_t, y_t, bp_sb)

                    stats = st_pool.tile([128, SG, 6], F32, name="stats", tag="stats")
                    for sg in range(SG):
                        nc.vector.bn_stats(out=stats[:, sg, :],
                                           in_=y_t[:, sg * 512:(sg + 1) * 512])
                    mv = st_pool.tile([128, 2], F32, name="mv", tag="mv")
                    nc.vector.bn_aggr(out=mv, in_=stats)
                    nc.scalar.activation(out=mv[:, 1:2], in_=mv[:, 1:2],
                                         func=AF.Sqrt, bias=eps_t)
                    nc.vector.reciprocal(mv[:, 1:2], mv[:, 1:2])
                    nc.vector.tensor_scalar(
                        out=y_t, in0=y_t,
                        scalar1=mv[:, 0:1], scalar2=mv[:, 1:2],
                        op0=mybir.AluOpType.subtract, op1=mybir.AluOpType.mult,
                    )
                    if has_gamma:
                        nc.vector.tensor_mul(y_t, y_t, g_sb)
                    if has_beta:
                        nc.vector.tensor_add(y_t, y_t, be_sb)
                    nc.sync.dma_start(
                        out=out.ap()[ts0 + tsub * 128:ts0 + (tsub + 1) * 128, :],
                        in_=y_t)

    nc.compile()
    return nc


_CACHE = {}


def _get_nc(flags):
    if flags not in _CACHE:
        _CACHE[flags] = _build(*flags)
    return _CACHE[flags]


def kernel(x, Wqkv, bqkv, Wproj, bproj, gamma, beta):
    x = np.ascontiguousarray(np.asarray(x, dtype=np.float32))
    Wqkv = np.asarray(Wqkv, dtype=np.float32)
    bqkv = np.asarray(bqkv, dtype=np.float32)
    Wproj = np.asarray(Wproj, dtype=np.float32)
    bproj = np.asarray(bproj, dtype=np.float32)
    gamma = np.asarray(gamma, dtype=np.float32)
    beta = np.asarray(beta, dtype=np.float32)

    scale = np.float32(HD ** -0.5)
    # fold the attention scale into Wq / bq
    svec = np.ones((2 * D,), np.float32)
    svec[:D] = scale
    wqk_h = np.ascontiguousarray((Wqkv[:2 * D] * svec[:, None]).T)   # [D, 2D]
    bqk_full = bqkv[:2 * D] * svec
    # V weights with a zero column per head (ones come from an on-chip memset)
    wv_h = np.zeros((D, VC), np.float32)
    bv_full = np.zeros((VC,), np.float32)
    wvT = Wqkv[2 * D:].T                                             # [D, D]
    for h in range(H):
        wv_h[:, h * (HD + 1):h * (HD + 1) + HD] = wvT[:, h * HD:(h + 1) * HD]
        bv_full[h * (HD + 1):h * (HD + 1) + HD] = bqkv[2 * D + h * HD:2 * D + (h + 1) * HD]
    wp_h = np.ascontiguousarray(Wproj.T)                             # [D, D]

    has_bqk = bool(np.any(bqk_full))
    has_bv = bool(np.any(bv_full))
    has_bp = bool(np.any(bproj))
    has_gamma = not bool(np.all(gamma == 1.0))
    has_beta = bool(np.any(beta))
    nc = _get_nc((has_bqk, has_bv, has_bp, has_gamma, has_beta))

    xf = x.reshape(B * L, D)
    in_maps = []
    for c in range(NCORES):
        sl = slice(c * TOK, (c + 1) * TOK)
        m = {
            "xT": np.ascontiguousarray(xf[sl].T),
            "xr": np.ascontiguousarray(xf[sl]),
            "wqk": wqk_h,
            "wv": wv_h,
            "wp": wp_h,
        }
        if has_bqk:
            m["bqk"] = np.ascontiguousarray(bqk_full.reshape(2 * KT, 128).T)
        if has_bv:
            m["bv"] = bv_full.reshape(1, VC)
        if has_bp:
            m["bp"] = bproj.reshape(1, D)
        if has_gamma:
            m["gamma"] = gamma.reshape(1, D)
        if has_beta:
            m["beta"] = beta.reshape(1, D)
        in_maps.append(m)

    trace = bool(os.environ.get("KERNEL_TRACE"))
    if trace:
        _maybe_install_ntff_hook()
    res = run_bass_kernel_spmd(nc, in_maps, core_ids=list(range(NCORES)),
                               trace=trace)
    if trace:
        kernel.last_exec_time_ns = res.exec_time_ns
        kernel.last_mean_exec_time_ns = res.mean_exec_time_ns
        kernel.last_results = res

    y = np.concatenate([res.results[c]["out"] for c in range(NCORES)], axis=0)
    return y.reshape(B, L, D)
